# revision 1
# baseline (speedup 1.0000x reference)
"""Causal self-attention (B=2, N=2048, D=2048, H=16, hd=128) on 8 Trainium2
NeuronCores.

Strategy (tensor-parallel over heads, 2 heads/core):
  - Host: transpose x / weights, build RoPE tables + triangular mask consts,
    slice w_qkv rows per head-group.
  - Device, per core (same SPMD program, different input data):
    Phase A: qkvT projection (f32r matmuls, outputs in [d, n] layout) + RoPE
             (partition-rotate via SBUF DMA + DVE mul/add).
    Phase B: attention fully in transposed orientation: S.T = kT.T @ qT
             (PE), P.T = exp(S.T) (ACT), causal mask via sliced triangular
             const (DVE), O.T accumulated as vT.T @ P.T (PE, PSUM accum).
             Softmax denominators: DVE-accumulate P.T tiles, ones-matmul to
             reduce over partitions, reciprocal, ones-bcast matmul, scale.
    AllToAll: reshard O.T from head-sharded to row-sharded (full inner dim).
    Phase C: o_proj on the 512-row shard: out = O.T_full.T @ w_o.T.
  - Host: concatenate the 8 row-shards.

Zero on-device transposes: every matmul consumes operands in the layout the
previous phase produced.
"""

import sys
import time

import ml_dtypes
import numpy as np

sys.path.insert(0, "/opt/trn_rl_repo")

import concourse.bacc as bacc  # noqa: E402
import concourse.bass as bass  # noqa: E402
import concourse.mybir as mybir  # noqa: E402
import concourse.tile as tile  # noqa: E402
from concourse import bass_utils  # noqa: E402

F32 = mybir.dt.float32
BF16 = mybir.dt.bfloat16

B, N, D = 2, 2048, 2048
H, HD = 16, 128
NC = 8
HPC = H // NC          # heads per core
BN = B * N             # 4096
NSH = BN // NC         # output rows per core
INNER = H * HD
ROPE_BASE = 10000.0

_CACHE = {}

LAST_EXEC_NS = None
LAST_RESULTS = None


def _build_program():
    nc = bacc.Bacc(
        "TRN2",
        target_bir_lowering=False,
        debug=False,
        enable_asserts=False,
        num_devices=NC,
    )
    xT = nc.dram_tensor("xT", [D, BN], BF16, kind="ExternalInput").ap()
    wqkT = nc.dram_tensor("wqkT", [D, 4 * HD], BF16, kind="ExternalInput").ap()
    wvT = nc.dram_tensor("wvT", [D, HPC * HD], BF16, kind="ExternalInput").ap()
    woT = nc.dram_tensor("woT", [INNER, D], BF16, kind="ExternalInput").ap()
    tabs = nc.dram_tensor("tabs", [4, HD, BN], BF16, kind="ExternalInput").ap()
    tri = nc.dram_tensor("tri", [128, 1024], BF16, kind="ExternalInput").ap()
    out = nc.dram_tensor("out", [NSH, D], F32, kind="ExternalOutput").ap()
    a2a_in = nc.dram_tensor("a2a_in", [NC, HPC, 128, 512], BF16).ap()
    a2a_out = nc.dram_tensor("a2a_out", [NC, HPC, 128, 512], BF16).ap()

    MUL = mybir.AluOpType.mult
    ADD = mybir.AluOpType.add
    SUB = mybir.AluOpType.subtract
    EXP = mybir.ActivationFunctionType.Exp

    with tile.TileContext(nc, num_cores=NC) as tc:
        with (
            tc.tile_pool(name="const", bufs=1) as constp,
            tc.tile_pool(name="wqk", bufs=1) as wqkp,
            tc.tile_pool(name="wv", bufs=1) as wvp,
            tc.tile_pool(name="persist", bufs=1) as persist,
        ):
            tri_sb = constp.tile([128, 1024], BF16, name="tri_sb")
            nc.sync.dma_start(out=tri_sb[:, :], in_=tri[:, :])
            ones_col = constp.tile([128, 1], F32, name="ones_col")
            nc.vector.memset(ones_col[:, :], 1.0)
            ones_row = constp.tile([1, 128], F32, name="ones_row")
            nc.vector.memset(ones_row[:, :], 1.0)
            wqk_sb = wqkp.tile([128, 16, 512], BF16, name="wqk_sb")
            nc.sync.dma_start(
                out=wqk_sb[:, :, :],
                in_=wqkT.rearrange("(k p) m -> p k m", p=128),
            )
            wv_sb = wvp.tile([128, 16, 256], BF16, name="wv_sb")
            nc.sync.dma_start(
                out=wv_sb[:, :, :],
                in_=wvT.rearrange("(k p) m -> p k m", p=128),
            )

            with (
                tc.tile_pool(name="xt", bufs=3) as xtp,
                tc.tile_pool(name="tab", bufs=2) as tabp,
                tc.tile_pool(name="rope", bufs=2) as ropep,
                tc.tile_pool(name="pt", bufs=3) as ptp,
                tc.tile_pool(name="rs", bufs=2) as rsp,
                tc.tile_pool(name="small", bufs=2) as smallp,
                tc.tile_pool(name="ots", bufs=2) as otsp,
                tc.tile_pool(name="pst", bufs=3, space="PSUM") as pstp,
                tc.tile_pool(name="pov", bufs=3, space="PSUM") as povp,
                tc.tile_pool(name="psmall", bufs=1, space="PSUM") as psmallp,
            ):
                for b in range(B):
                    qkT_sb = persist.tile(
                        [128, 4, N], BF16, tag="qkT", name=f"qkT_b{b}"
                    )
                    vT_sb = persist.tile(
                        [128, 16, HPC * HD], BF16, tag="vT", name=f"vT_b{b}"
                    )
                    # ---------------- phase A: projection + RoPE ----------
                    for j in range(4):
                        n0 = b * N + 512 * j
                        xh = []
                        for half in range(2):
                            t = xtp.tile(
                                [128, 8, 512], BF16, tag="xt", name=f"xt_{b}_{j}_{half}"
                            )
                            nc.sync.dma_start(
                                out=t[:, :, :],
                                in_=xT.rearrange("(k p) n -> p k n", p=128)[
                                    :, 8 * half : 8 * half + 8, n0 : n0 + 512
                                ],
                            )
                            xh.append(t)
                        tabt = []
                        for ti in range(4):
                            tt = tabp.tile([128, 512], BF16, tag=f"tab{ti}", name=f"tab{ti}_{b}_{j}")
                            nc.sync.dma_start(out=tt[:, :], in_=tabs[ti, :, n0 : n0 + 512])
                            tabt.append(tt)
                        for pair in (0, 2):
                            psA = pstp.tile([128, 512], F32, tag="pst", name=f"psA_{b}_{j}_{pair}")
                            psB = pstp.tile([128, 512], F32, tag="pst", name=f"psB_{b}_{j}_{pair}")
                            for mt, pst_ in ((pair, psA), (pair + 1, psB)):
                                for k in range(16):
                                    nc.tensor.matmul(
                                        pst_[:, :],
                                        lhsT=(wqk_sb[:, k, 128 * mt : 128 * mt + 128]),
                                        rhs=(xh[k // 8][:, k % 8, :]),
                                        start=(k == 0),
                                        stop=(k == 15),
                                    )
                            ci = 0 if pair == 0 else 2
                            t1 = ropep.tile([128, 512], BF16, tag="t1", name=f"t1_{b}_{j}_{pair}")
                            t2 = ropep.tile([128, 512], BF16, tag="t2", name=f"t2_{b}_{j}_{pair}")
                            t3 = ropep.tile([128, 512], BF16, tag="t3", name=f"t3_{b}_{j}_{pair}")
                            t4 = ropep.tile([128, 512], BF16, tag="t4", name=f"t4_{b}_{j}_{pair}")
                            nc.vector.tensor_tensor(t1[:, :], psA[:, :], tabt[ci][:, :], MUL)
                            nc.vector.tensor_tensor(t2[:, :], psB[:, :], tabt[ci + 1][:, :], MUL)
                            nc.vector.tensor_tensor(t3[:, :], psB[:, :], tabt[ci][:, :], MUL)
                            nc.vector.tensor_tensor(t4[:, :], psA[:, :], tabt[ci + 1][:, :], MUL)
                            nc.vector.tensor_tensor(
                                qkT_sb[:, pair, 512 * j : 512 * (j + 1)], t1[:, :], t2[:, :], SUB
                            )
                            nc.vector.tensor_tensor(
                                qkT_sb[:, pair + 1, 512 * j : 512 * (j + 1)], t3[:, :], t4[:, :], ADD
                            )
                        for mt in range(4):
                            pv = povp.tile([128, 256], F32, tag="pov", name=f"psV_{b}_{j}_{mt}")
                            for k in range(16):
                                nc.tensor.matmul(
                                    pv[:, :],
                                    lhsT=(xh[k // 8][:, k % 8, 128 * mt : 128 * mt + 128]),
                                    rhs=(wv_sb[:, k, :]),
                                    start=(k == 0),
                                    stop=(k == 15),
                                )
                            nc.scalar.copy(vT_sb[:, 4 * j + mt, :], pv[:, :])
                    # ---------------- phase B: attention ------------------
                    for h in range(HPC):
                        for j in range(4):
                            ov = povp.tile([128, 512], F32, tag="pov", name=f"ov_{b}_{h}_{j}")
                            rs_c = rsp.tile([128, 512], F32, tag="rs", name=f"rs_{b}_{h}_{j}")
                            nc.vector.memset(rs_c[:, :], 0.0)
                            for t in range(4 * j + 4):
                                st = pstp.tile(
                                    [128, 512], F32, tag="pst", name=f"st_{b}_{h}_{j}_{t}"
                                )
                                nc.tensor.matmul(
                                    st[:, :],
                                    lhsT=(qkT_sb[64 * h : 64 * h + 64, 2, 128 * t : 128 * t + 128]),
                                    rhs=(qkT_sb[64 * h : 64 * h + 64, 0, 512 * j : 512 * (j + 1)]),
                                    start=True,
                                    stop=False,
                                )
                                nc.tensor.matmul(
                                    st[:, :],
                                    lhsT=(qkT_sb[64 * h : 64 * h + 64, 3, 128 * t : 128 * t + 128]),
                                    rhs=(qkT_sb[64 * h : 64 * h + 64, 1, 512 * j : 512 * (j + 1)]),
                                    start=False,
                                    stop=True,
                                )
                                pt = ptp.tile(
                                    [128, 512], BF16, tag="pt", name=f"pt_{b}_{h}_{j}_{t}"
                                )
                                nc.scalar.activation(pt[:, :], st[:, :], EXP)
                                if t // 4 == j:
                                    f0 = 128 * t - 512 * j
                                    nc.vector.tensor_tensor(
                                        pt[:, :], pt[:, :],
                                        tri_sb[:, 512 - f0 : 1024 - f0], MUL,
                                    )
                                nc.vector.tensor_tensor(rs_c[:, :], rs_c[:, :], pt[:, :], ADD)
                                nc.tensor.matmul(
                                    ov[:, :],
                                    lhsT=(vT_sb[:, t, 128 * h : 128 * h + 128]),
                                    rhs=(pt[:, :]),
                                    start=(t == 0),
                                    stop=(t == 4 * j + 3),
                                )
                            rsum = psmallp.tile([1, 512], F32, tag="rsum", name=f"rsum_{b}_{h}_{j}")
                            nc.tensor.matmul(
                                rsum[:, :], lhsT=ones_col[:, :], rhs=rs_c[:, :],
                                start=True, stop=True,
                            )
                            rinv = smallp.tile([1, 512], F32, tag="rinv", name=f"rinv_{b}_{h}_{j}")
                            nc.vector.reciprocal(rinv[:, :], rsum[:, :])
                            binv = psmallp.tile([128, 512], F32, tag="binv", name=f"binv_{b}_{h}_{j}")
                            nc.tensor.matmul(
                                binv[:, :], lhsT=ones_row[:, :], rhs=rinv[:, :],
                                start=True, stop=True,
                            )
                            binv_sb = smallp.tile(
                                [128, 512], F32, tag="binv_sb", name=f"binvs_{b}_{h}_{j}"
                            )
                            nc.scalar.copy(binv_sb[:, :], binv[:, :])
                            ot = otsp.tile([128, 512], BF16, tag="ot", name=f"ot_{b}_{h}_{j}")
                            nc.vector.tensor_tensor(ot[:, :], ov[:, :], binv_sb[:, :], MUL)
                            nc.sync.dma_start(
                                out=a2a_in[4 * b + j, h, :, :], in_=ot[:, :]
                            )

            # ---------------- AllToAll reshard ----------------------------
            nc.gpsimd.collective_compute(
                "AllToAll",
                mybir.AluOpType.bypass,
                replica_groups=[list(range(NC))],
                ins=[a2a_in.opt()],
                outs=[a2a_out.opt()],
            )

            # ---------------- phase C: o_proj ------------------------------
            with (
                tc.tile_pool(name="opin", bufs=1) as opinp,
                tc.tile_pool(name="wo", bufs=4) as wop,
                tc.tile_pool(name="outs", bufs=4) as outsp,
                tc.tile_pool(name="pc", bufs=4, space="PSUM") as pcp,
            ):
                opin = opinp.tile([128, 16, 512], BF16, name="opin")
                nc.sync.dma_start(
                    out=opin[:, :, :],
                    in_=a2a_out.rearrange("r h p n -> p (r h) n"),
                )
                for dc in range(4):
                    pcs = [
                        pcp.tile([128, 512], F32, tag="pc", name=f"pc_{dc}_{ns}")
                        for ns in range(4)
                    ]
                    for k in range(16):
                        wo_t = wop.tile([128, 512], BF16, tag="wo", name=f"wo_{dc}_{k}")
                        nc.sync.dma_start(
                            out=wo_t[:, :],
                            in_=woT.rearrange("(k p) d -> p k d", p=128)[
                                :, k, 512 * dc : 512 * (dc + 1)
                            ],
                        )
                        for ns in range(4):
                            nc.tensor.matmul(
                                pcs[ns][:, :],
                                lhsT=(opin[:, k, 128 * ns : 128 * ns + 128]),
                                rhs=(wo_t[:, :]),
                                start=(k == 0),
                                stop=(k == 15),
                            )
                    for ns in range(4):
                        ost = outsp.tile([128, 512], F32, tag="outs", name=f"os_{dc}_{ns}")
                        nc.scalar.copy(ost[:, :], pcs[ns][:, :])
                        nc.sync.dma_start(
                            out=out[128 * ns : 128 * (ns + 1), 512 * dc : 512 * (dc + 1)],
                            in_=ost[:, :],
                        )
    nc.compile()
    return nc


def _host_prep(x, w_qkv, w_o):
    bf = ml_dtypes.bfloat16
    xT = np.ascontiguousarray(x.reshape(BN, D).T).astype(bf)
    woT = np.ascontiguousarray(np.asarray(w_o).T).astype(bf)

    inv_freq = 1.0 / (ROPE_BASE ** (np.arange(0, HD, 2, dtype=np.float32) / HD))
    ang = np.arange(N, dtype=np.float32)[:, None] * inv_freq[None, :]
    cos_h = np.cos(ang).T.astype(np.float32)      # [64, N]
    sin_h = np.sin(ang).T.astype(np.float32)      # [64, N] (magnitude)
    # duplicated for the two heads packed per 128-row block
    cos2 = np.concatenate([cos_h, cos_h], axis=0)  # [128, N]
    sin2 = np.concatenate([sin_h, sin_h], axis=0)
    cos_f = np.tile(cos2, (1, B))
    sin_f = np.tile(sin2, (1, B))
    scale = np.float32(1.0 / np.sqrt(HD))
    tabs = np.ascontiguousarray(
        np.stack([cos_f * scale, sin_f * scale, cos_f, sin_f], axis=0)
    ).astype(bf)

    p = np.arange(128)[:, None]
    c = np.arange(1024)[None, :]
    tri = (p <= c - 512).astype(bf)

    in_maps = []
    for core in range(NC):
        h0 = core * HPC
        rq = slice(h0 * HD, (h0 + HPC) * HD)
        rk = slice(INNER + h0 * HD, INNER + (h0 + HPC) * HD)
        rv = slice(2 * INNER + h0 * HD, 2 * INNER + (h0 + HPC) * HD)
        wq = w_qkv[rq].reshape(HPC, HD, D)
        wk = w_qkv[rk].reshape(HPC, HD, D)
        # row order per block: [h0_lo, h1_lo | h0_hi, h1_hi] for q then k
        wqkT = np.ascontiguousarray(
            np.concatenate(
                [wq[0, :64], wq[1, :64], wq[0, 64:], wq[1, 64:],
                 wk[0, :64], wk[1, :64], wk[0, 64:], wk[1, 64:]], axis=0
            ).T
        ).astype(bf)
        wvT = np.ascontiguousarray(w_qkv[rv].T).astype(bf)
        in_maps.append(
            dict(xT=xT, wqkT=wqkT, wvT=wvT, woT=woT, tabs=tabs, tri=tri)
        )
    return in_maps


def kernel(x, w_qkv, w_o, n_heads=None, head_dim=None, trace=False):
    global LAST_EXEC_NS, LAST_RESULTS
    x = np.asarray(x, dtype=np.float32)
    w_qkv = np.asarray(w_qkv, dtype=np.float32)
    w_o = np.asarray(w_o, dtype=np.float32)

    if "nc" not in _CACHE:
        _CACHE["nc"] = _build_program()
    nc = _CACHE["nc"]

    in_maps = _host_prep(x, w_qkv, w_o)
    res = None
    last_exc = None
    for attempt in range(4):
        try:
            res = bass_utils.run_bass_kernel_spmd(
                nc, in_maps, core_ids=list(range(NC)), trace=trace
            )
            break
        except Exception as e:  # transient compile_and_load / exec flakiness
            last_exc = e
            print(f"kernel attempt {attempt} failed: {e}", file=sys.stderr)
            time.sleep(5)
    if res is None:
        raise last_exc
    LAST_EXEC_NS = res.exec_time_ns
    LAST_RESULTS = res
    shards = [res.results[c]["out"] for c in range(NC)]
    full = np.concatenate(shards, axis=0).reshape(B, N, D).astype(np.float32)
    return full



# revision 4
# speedup vs baseline: 1.3879x; 1.3879x over previous
"""Causal self-attention (B=2, N=2048, D=2048, H=16, hd=128) on 8 Trainium2
NeuronCores.

Strategy (tensor-parallel over heads, 2 heads/core), v2:
  - Host: transpose x / weights, build RoPE tables + triangular mask consts,
    slice w_qkv rows per head-group.
  - Device, per core (same SPMD program, different input data):
    Phase A: qkvT projection (bf16 matmuls, outputs in [d, n] layout) + RoPE
             (DVE mul/add on psum pairs) -> stage tiles -> SBUF->SBUF DMA
             repack into per-head [128=hd, N] q/k tiles (full-contract
             scores).
    Phase B: S.T = kh.T @ qh in ONE c=128 matmul per tile, P.T = exp(S.T)
             (ACT), causal mask via sliced triangular const (DVE), O.T
             accumulated as vT.T @ P.T (PE, PSUM accum).  Softmax denoms:
             ones-column matmul accumulated in PSUM over t (PE), fast
             reciprocal (DVE custom op), partition_broadcast (GPSIMD),
             final scale (DVE).
    Per-batch AllToAll (256-row chunks to each core) fired right after each
    batch's attention: b0's collective hides under b1's compute.
    Phase C: o_proj on the 2x256-row shard with w_o pre-cached in SBUF
             (8MB DMA issued during b0's attention); first half overlaps
             b1's collective.
  - Host: reassemble [b0 rows 256c:256c+256 | b1 rows 256c:256c+256].
"""

import sys
import time

import ml_dtypes
import numpy as np

sys.path.insert(0, "/opt/trn_rl_repo")

import concourse.bacc as bacc  # noqa: E402
import concourse.bass as bass  # noqa: E402
import concourse.mybir as mybir  # noqa: E402
import concourse.tile as tile  # noqa: E402
from concourse import bass_utils  # noqa: E402

F32 = mybir.dt.float32
BF16 = mybir.dt.bfloat16

B, N, D = 2, 2048, 2048
H, HD = 16, 128
NC = 8
HPC = H // NC          # heads per core
BN = B * N             # 4096
NSH = BN // NC         # output rows per core
INNER = H * HD
ROPE_BASE = 10000.0

_CACHE = {}

LAST_EXEC_NS = None
LAST_RESULTS = None


def _build_program():
    nc = bacc.Bacc(
        "TRN2",
        target_bir_lowering=False,
        debug=False,
        enable_asserts=False,
        num_devices=NC,
    )
    xT = nc.dram_tensor("xT", [D, BN], BF16, kind="ExternalInput").ap()
    wqkT = nc.dram_tensor("wqkT", [D, 4 * HD], BF16, kind="ExternalInput").ap()
    wvT = nc.dram_tensor("wvT", [D, HPC * HD], BF16, kind="ExternalInput").ap()
    woT = nc.dram_tensor("woT", [INNER, D], BF16, kind="ExternalInput").ap()
    tabs = nc.dram_tensor("tabs", [4, HD, BN], BF16, kind="ExternalInput").ap()
    tri = nc.dram_tensor("tri", [128, 1024], BF16, kind="ExternalInput").ap()
    out = nc.dram_tensor("out", [NSH, D], F32, kind="ExternalOutput").ap()
    a2a_in = [
        nc.dram_tensor(f"a2a_in{b}", [NC, HPC, 128, 256], BF16).ap()
        for b in range(B)
    ]
    a2a_out = [
        nc.dram_tensor(f"a2a_out{b}", [NC, HPC, 128, 256], BF16).ap()
        for b in range(B)
    ]

    MUL = mybir.AluOpType.mult
    ADD = mybir.AluOpType.add
    SUB = mybir.AluOpType.subtract
    EXP = mybir.ActivationFunctionType.Exp

    with tile.TileContext(nc, num_cores=NC) as tc:
        with (
            tc.tile_pool(name="const", bufs=1) as constp,
            tc.tile_pool(name="wqk", bufs=1) as wqkp,
            tc.tile_pool(name="wv", bufs=1) as wvp,
            tc.tile_pool(name="wo", bufs=1) as wop,
            tc.tile_pool(name="persist", bufs=2) as persist,
        ):
            tri_sb = constp.tile([128, 1024], BF16, name="tri_sb")
            nc.sync.dma_start(out=tri_sb[:, :], in_=tri[:, :])
            ones_col = constp.tile([128, 1], BF16, name="ones_col")
            nc.vector.memset(ones_col[:, :], 1.0)
            wqk_sb = wqkp.tile([128, 16, 512], BF16, name="wqk_sb")
            nc.sync.dma_start(
                out=wqk_sb[:, :, :],
                in_=wqkT.rearrange("(k p) m -> p k m", p=128),
            )
            wv_sb = wvp.tile([128, 16, 256], BF16, name="wv_sb")
            nc.sync.dma_start(
                out=wv_sb[:, :, :],
                in_=wvT.rearrange("(k p) m -> p k m", p=128),
            )
            wo_sb = wop.tile([128, 16, D], BF16, name="wo_sb")

            with (
                tc.tile_pool(name="xt", bufs=3) as xtp,
                tc.tile_pool(name="tab", bufs=2) as tabp,
                tc.tile_pool(name="rope", bufs=2) as ropep,
                tc.tile_pool(name="stage", bufs=3) as stagep,
                tc.tile_pool(name="pt", bufs=4) as ptp,
                tc.tile_pool(name="small", bufs=2) as smallp,
                tc.tile_pool(name="ots", bufs=2) as otsp,
                tc.tile_pool(name="pst", bufs=3, space="PSUM") as pstp,
                tc.tile_pool(name="pov", bufs=3, space="PSUM") as povp,
                tc.tile_pool(name="psmall", bufs=2, space="PSUM") as psmallp,
            ):
                for b in range(B):
                    qh_sb = persist.tile(
                        [128, 4, N], BF16, tag="qh", name=f"qh_b{b}"
                    )
                    vT_sb = persist.tile(
                        [128, 16, HPC * HD], BF16, tag="vT", name=f"vT_b{b}"
                    )
                    # ---------------- phase A: projection + RoPE ----------
                    for j in range(4):
                        n0 = b * N + 512 * j
                        xh = []
                        for half in range(2):
                            t = xtp.tile(
                                [128, 8, 512], BF16, tag="xt", name=f"xt_{b}_{j}_{half}"
                            )
                            nc.sync.dma_start(
                                out=t[:, :, :],
                                in_=xT.rearrange("(k p) n -> p k n", p=128)[
                                    :, 8 * half : 8 * half + 8, n0 : n0 + 512
                                ],
                            )
                            xh.append(t)
                        tabt = []
                        for ti in range(4):
                            tt = tabp.tile([128, 512], BF16, tag=f"tab{ti}", name=f"tab{ti}_{b}_{j}")
                            nc.sync.dma_start(out=tt[:, :], in_=tabs[ti, :, n0 : n0 + 512])
                            tabt.append(tt)
                        for pair in (0, 2):
                            psA = pstp.tile([128, 512], F32, tag="pst", name=f"psA_{b}_{j}_{pair}")
                            psB = pstp.tile([128, 512], F32, tag="pst", name=f"psB_{b}_{j}_{pair}")
                            for mt, pst_ in ((pair, psA), (pair + 1, psB)):
                                for k in range(16):
                                    nc.tensor.matmul(
                                        pst_[:, :],
                                        lhsT=(wqk_sb[:, k, 128 * mt : 128 * mt + 128]),
                                        rhs=(xh[k // 8][:, k % 8, :]),
                                        start=(k == 0),
                                        stop=(k == 15),
                                    )
                            ci = 0 if pair == 0 else 2
                            t1 = ropep.tile([128, 512], BF16, tag="t1", name=f"t1_{b}_{j}_{pair}")
                            t2 = ropep.tile([128, 512], BF16, tag="t2", name=f"t2_{b}_{j}_{pair}")
                            t3 = ropep.tile([128, 512], BF16, tag="t3", name=f"t3_{b}_{j}_{pair}")
                            t4 = ropep.tile([128, 512], BF16, tag="t4", name=f"t4_{b}_{j}_{pair}")
                            nc.vector.tensor_tensor(t1[:, :], psA[:, :], tabt[ci][:, :], MUL)
                            nc.vector.tensor_tensor(t2[:, :], psB[:, :], tabt[ci + 1][:, :], MUL)
                            nc.vector.tensor_tensor(t3[:, :], psB[:, :], tabt[ci][:, :], MUL)
                            nc.vector.tensor_tensor(t4[:, :], psA[:, :], tabt[ci + 1][:, :], MUL)
                            sl = stagep.tile([128, 512], BF16, tag="sl", name=f"sl_{b}_{j}_{pair}")
                            sh = stagep.tile([128, 512], BF16, tag="sh", name=f"sh_{b}_{j}_{pair}")
                            nc.vector.tensor_tensor(sl[:, :], t1[:, :], t2[:, :], SUB)
                            nc.vector.tensor_tensor(sh[:, :], t3[:, :], t4[:, :], ADD)
                            # repack: per-head [lo;hi] tiles for full-contract
                            # scores.  base tile index: q -> 0, k -> 2.
                            base = 0 if pair == 0 else 2
                            cs = slice(512 * j, 512 * (j + 1))
                            nc.sync.dma_start(out=qh_sb[0:64, base, cs], in_=sl[0:64, :])
                            nc.sync.dma_start(out=qh_sb[0:64, base + 1, cs], in_=sl[64:128, :])
                            nc.sync.dma_start(out=qh_sb[64:128, base, cs], in_=sh[0:64, :])
                            nc.sync.dma_start(out=qh_sb[64:128, base + 1, cs], in_=sh[64:128, :])
                        for mt in range(4):
                            pv = povp.tile([128, 256], F32, tag="pov", name=f"psV_{b}_{j}_{mt}")
                            for k in range(16):
                                nc.tensor.matmul(
                                    pv[:, :],
                                    lhsT=(xh[k // 8][:, k % 8, 128 * mt : 128 * mt + 128]),
                                    rhs=(wv_sb[:, k, :]),
                                    start=(k == 0),
                                    stop=(k == 15),
                                )
                            nc.scalar.copy(vT_sb[:, 4 * j + mt, :], pv[:, :])

                    if b == 0:
                        # w_o cache fill: issued here so the 8MB transfer
                        # rides under b0's attention, clear of x loads.
                        nc.sync.dma_start(
                            out=wo_sb[:, :, :],
                            in_=woT.rearrange("(k p) d -> p k d", p=128),
                        )

                    # ---------------- phase B: attention ------------------
                    for h in range(HPC):
                        for j in range(4):
                            ov = povp.tile([128, 512], F32, tag="pov", name=f"ov_{b}_{h}_{j}")
                            rsum = psmallp.tile(
                                [1, 512], F32, tag="rsum", name=f"rsum_{b}_{h}_{j}"
                            )
                            nt = 4 * j + 4
                            for t in range(nt):
                                st = pstp.tile(
                                    [128, 512], F32, tag="pst", name=f"st_{b}_{h}_{j}_{t}"
                                )
                                nc.tensor.matmul(
                                    st[:, :],
                                    lhsT=(qh_sb[:, 2 + h, 128 * t : 128 * t + 128]),
                                    rhs=(qh_sb[:, h, 512 * j : 512 * (j + 1)]),
                                    start=True,
                                    stop=True,
                                )
                                pt = ptp.tile(
                                    [128, 512], BF16, tag="pt", name=f"pt_{b}_{h}_{j}_{t}"
                                )
                                nc.scalar.activation(pt[:, :], st[:, :], EXP)
                                if t // 4 == j:
                                    f0 = 128 * t - 512 * j
                                    nc.vector.tensor_tensor(
                                        pt[:, :], pt[:, :],
                                        tri_sb[:, 512 - f0 : 1024 - f0], MUL,
                                    )
                                nc.tensor.matmul(
                                    rsum[:, :],
                                    lhsT=ones_col[:, :],
                                    rhs=pt[:, :],
                                    start=(t == 0),
                                    stop=(t == nt - 1),
                                )
                                nc.tensor.matmul(
                                    ov[:, :],
                                    lhsT=(vT_sb[:, t, 128 * h : 128 * h + 128]),
                                    rhs=(pt[:, :]),
                                    start=(t == 0),
                                    stop=(t == nt - 1),
                                )
                            rinv = smallp.tile([1, 512], F32, tag="rinv", name=f"rinv_{b}_{h}_{j}")
                            nc.vector.reciprocal_approx_fast(rinv[:, :], rsum[:, :])
                            binv = smallp.tile(
                                [128, 512], F32, tag="binv", name=f"binv_{b}_{h}_{j}"
                            )
                            nc.gpsimd.partition_broadcast(binv[:, :], rinv[:, :])
                            ot = otsp.tile([128, 512], BF16, tag="ot", name=f"ot_{b}_{h}_{j}")
                            nc.vector.tensor_tensor(ot[:, :], ov[:, :], binv[:, :], MUL)
                            nc.sync.dma_start(
                                out=a2a_in[b][2 * j, h, :, :], in_=ot[:, 0:256]
                            )
                            nc.sync.dma_start(
                                out=a2a_in[b][2 * j + 1, h, :, :], in_=ot[:, 256:512]
                            )

                    # per-batch reshard: b0's collective hides under b1's
                    # compute; b1's hides under phase C's first half.
                    nc.gpsimd.collective_compute(
                        "AllToAll",
                        mybir.AluOpType.bypass,
                        replica_groups=[list(range(NC))],
                        ins=[a2a_in[b].opt()],
                        outs=[a2a_out[b].opt()],
                    )

            # ---------------- phase C: o_proj ------------------------------
            with (
                tc.tile_pool(name="opin", bufs=1) as opinp,
                tc.tile_pool(name="outs", bufs=4) as outsp,
                tc.tile_pool(name="pc", bufs=4, space="PSUM") as pcp,
            ):
                for half in range(B):
                    opin = opinp.tile([128, 16, 256], BF16, tag=f"opin{half}", name=f"opin{half}")
                    nc.sync.dma_start(
                        out=opin[:, :, :],
                        in_=a2a_out[half].rearrange("r h p n -> p (r h) n"),
                    )
                    for dc in range(4):
                        for ns in range(2):
                            pc = pcp.tile([128, 512], F32, tag="pc", name=f"pc_{half}_{dc}_{ns}")
                            for k in range(16):
                                nc.tensor.matmul(
                                    pc[:, :],
                                    lhsT=(opin[:, k, 128 * ns : 128 * ns + 128]),
                                    rhs=(wo_sb[:, k, 512 * dc : 512 * (dc + 1)]),
                                    start=(k == 0),
                                    stop=(k == 15),
                                )
                            ost = outsp.tile([128, 512], F32, tag="outs", name=f"os_{half}_{dc}_{ns}")
                            nc.scalar.copy(ost[:, :], pc[:, :])
                            r0 = 256 * half + 128 * ns
                            nc.sync.dma_start(
                                out=out[r0 : r0 + 128, 512 * dc : 512 * (dc + 1)],
                                in_=ost[:, :],
                            )
    nc.compile()
    return nc


def _host_prep(x, w_qkv, w_o):
    bf = ml_dtypes.bfloat16
    xT = np.ascontiguousarray(x.reshape(BN, D).T).astype(bf)
    woT = np.ascontiguousarray(np.asarray(w_o).T).astype(bf)

    inv_freq = 1.0 / (ROPE_BASE ** (np.arange(0, HD, 2, dtype=np.float32) / HD))
    ang = np.arange(N, dtype=np.float32)[:, None] * inv_freq[None, :]
    cos_h = np.cos(ang).T.astype(np.float32)      # [64, N]
    sin_h = np.sin(ang).T.astype(np.float32)      # [64, N] (magnitude)
    # duplicated for the two heads packed per 128-row block
    cos2 = np.concatenate([cos_h, cos_h], axis=0)  # [128, N]
    sin2 = np.concatenate([sin_h, sin_h], axis=0)
    cos_f = np.tile(cos2, (1, B))
    sin_f = np.tile(sin2, (1, B))
    scale = np.float32(1.0 / np.sqrt(HD))
    tabs = np.ascontiguousarray(
        np.stack([cos_f * scale, sin_f * scale, cos_f, sin_f], axis=0)
    ).astype(bf)

    p = np.arange(128)[:, None]
    c = np.arange(1024)[None, :]
    tri = (p <= c - 512).astype(bf)

    in_maps = []
    for core in range(NC):
        h0 = core * HPC
        rq = slice(h0 * HD, (h0 + HPC) * HD)
        rk = slice(INNER + h0 * HD, INNER + (h0 + HPC) * HD)
        rv = slice(2 * INNER + h0 * HD, 2 * INNER + (h0 + HPC) * HD)
        wq = w_qkv[rq].reshape(HPC, HD, D)
        wk = w_qkv[rk].reshape(HPC, HD, D)
        # row order per block: [h0_lo, h1_lo | h0_hi, h1_hi] for q then k
        wqkT = np.ascontiguousarray(
            np.concatenate(
                [wq[0, :64], wq[1, :64], wq[0, 64:], wq[1, 64:],
                 wk[0, :64], wk[1, :64], wk[0, 64:], wk[1, 64:]], axis=0
            ).T
        ).astype(bf)
        wvT = np.ascontiguousarray(w_qkv[rv].T).astype(bf)
        in_maps.append(
            dict(xT=xT, wqkT=wqkT, wvT=wvT, woT=woT, tabs=tabs, tri=tri)
        )
    return in_maps


def kernel(x, w_qkv, w_o, n_heads=None, head_dim=None, trace=False):
    global LAST_EXEC_NS, LAST_RESULTS
    x = np.asarray(x, dtype=np.float32)
    w_qkv = np.asarray(w_qkv, dtype=np.float32)
    w_o = np.asarray(w_o, dtype=np.float32)

    if "nc" not in _CACHE:
        _CACHE["nc"] = _build_program()
    nc = _CACHE["nc"]

    in_maps = _host_prep(x, w_qkv, w_o)
    res = None
    last_exc = None
    for attempt in range(4):
        try:
            res = bass_utils.run_bass_kernel_spmd(
                nc, in_maps, core_ids=list(range(NC)), trace=trace
            )
            break
        except Exception as e:  # transient compile_and_load / exec flakiness
            last_exc = e
            print(f"kernel attempt {attempt} failed: {e}", file=sys.stderr)
            time.sleep(5)
    if res is None:
        raise last_exc
    LAST_EXEC_NS = res.exec_time_ns
    LAST_RESULTS = res
    # core c returns [512, D]: rows 0:256 = batch0 rows 256c:256c+256,
    # rows 256:512 = batch1 rows 256c:256c+256.
    full = np.empty((B, N, D), dtype=np.float32)
    for c in range(NC):
        shard = res.results[c]["out"]
        full[0, 256 * c : 256 * c + 256] = shard[0:256]
        full[1, 256 * c : 256 * c + 256] = shard[256:512]
    return full


# revision 12
# speedup vs baseline: 1.5010x; 1.0815x over previous
"""Causal self-attention (B=2, N=2048, D=2048, H=16, hd=128) on 8 Trainium2
NeuronCores.

Strategy (tensor-parallel over heads, 2 heads/core), v2:
  - Host: transpose x / weights, build RoPE tables + triangular mask consts,
    slice w_qkv rows per head-group.
  - Device, per core (same SPMD program, different input data):
    Phase A: qkvT projection (bf16 matmuls, outputs in [d, n] layout) + RoPE
             (DVE mul/add on psum pairs) -> stage tiles -> SBUF->SBUF DMA
             repack into per-head [128=hd, N] q/k tiles (full-contract
             scores).
    Phase B: S.T = kh.T @ qh in ONE c=128 matmul per tile, P.T = exp(S.T)
             (ACT), causal mask via sliced triangular const (DVE), O.T
             accumulated as vT.T @ P.T (PE, PSUM accum).  Softmax denoms:
             ones-column matmul accumulated in PSUM over t (PE), fast
             reciprocal (DVE custom op), partition_broadcast (GPSIMD),
             final scale (DVE).
    Per-batch AllToAll (256-row chunks to each core) fired right after each
    batch's attention: b0's collective hides under b1's compute.
    Phase C: o_proj on the 2x256-row shard with w_o pre-cached in SBUF
             (8MB DMA issued during b0's attention); first half overlaps
             b1's collective.
  - Host: reassemble [b0 rows 256c:256c+256 | b1 rows 256c:256c+256].
"""

import sys
import time

import ml_dtypes
import numpy as np

sys.path.insert(0, "/opt/trn_rl_repo")

import concourse.bacc as bacc  # noqa: E402
import concourse.bass as bass  # noqa: E402
import concourse.mybir as mybir  # noqa: E402
import concourse.tile as tile  # noqa: E402
from concourse import bass_utils  # noqa: E402

F32 = mybir.dt.float32
BF16 = mybir.dt.bfloat16

B, N, D = 2, 2048, 2048
H, HD = 16, 128
NC = 8
HPC = H // NC          # heads per core
BN = B * N             # 4096
NSH = BN // NC         # output rows per core
INNER = H * HD
ROPE_BASE = 10000.0

_CACHE = {}

LAST_EXEC_NS = None
LAST_RESULTS = None


def _build_program():
    nc = bacc.Bacc(
        "TRN2",
        target_bir_lowering=False,
        debug=False,
        enable_asserts=False,
        num_devices=NC,
    )
    xT = nc.dram_tensor("xT", [D, BN], BF16, kind="ExternalInput").ap()
    wqkT = nc.dram_tensor("wqkT", [D, 4 * HD], BF16, kind="ExternalInput").ap()
    wvT = nc.dram_tensor("wvT", [D, HPC * HD], BF16, kind="ExternalInput").ap()
    woT = nc.dram_tensor("woT", [INNER, D], BF16, kind="ExternalInput").ap()
    tabs = nc.dram_tensor("tabs", [4, HD, BN], BF16, kind="ExternalInput").ap()
    tri = nc.dram_tensor("tri", [128, 1024], BF16, kind="ExternalInput").ap()
    out = nc.dram_tensor("out", [NSH, D], F32, kind="ExternalOutput").ap()
    a2a_in = [
        [
            nc.dram_tensor(f"a2a_in{b}_{h}", [NC, 128, 256], BF16).ap()
            for h in range(HPC)
        ]
        for b in range(B)
    ]
    a2a_out = [
        [
            nc.dram_tensor(f"a2a_out{b}_{h}", [NC, 128, 256], BF16).ap()
            for h in range(HPC)
        ]
        for b in range(B)
    ]

    MUL = mybir.AluOpType.mult
    ADD = mybir.AluOpType.add
    SUB = mybir.AluOpType.subtract
    EXP = mybir.ActivationFunctionType.Exp

    with tile.TileContext(nc, num_cores=NC) as tc:
        with (
            tc.tile_pool(name="const", bufs=1) as constp,
            tc.tile_pool(name="wqk", bufs=1) as wqkp,
            tc.tile_pool(name="wv", bufs=1) as wvp,
            tc.tile_pool(name="wo", bufs=1) as wop,
            tc.tile_pool(name="persist", bufs=1) as persist,
        ):
            wqk_sb = wqkp.tile([128, 16, 512], BF16, name="wqk_sb")
            wv_sb = wvp.tile([128, 16, 256], BF16, name="wv_sb")
            wo_sb = wop.tile([128, 16, D], BF16, name="wo_sb")
            tri_sb = constp.tile([128, 1024], BF16, name="tri_sb")
            ones_col = constp.tile([128, 1], BF16, name="ones_col")

            with (
                tc.tile_pool(name="xt", bufs=5) as xtp,
                tc.tile_pool(name="tab", bufs=2) as tabp,
                tc.tile_pool(name="rope", bufs=2) as ropep,
                tc.tile_pool(name="stage", bufs=3) as stagep,
                tc.tile_pool(name="pt", bufs=4) as ptp,
                tc.tile_pool(name="small", bufs=2) as smallp,
                tc.tile_pool(name="ots", bufs=2) as otsp,
                tc.tile_pool(name="pst", bufs=3, space="PSUM") as pstp,
                tc.tile_pool(name="pov", bufs=3, space="PSUM") as povp,
                tc.tile_pool(name="psmall", bufs=2, space="PSUM") as psmallp,
            ):
                def load_x(b, j):
                    n0 = b * N + 512 * j
                    xh = []
                    for half in range(2):
                        t = xtp.tile(
                            [128, 8, 512], BF16, tag="xt", name=f"xt_{b}_{j}_{half}"
                        )
                        nc.sync.dma_start(
                            out=t[:, :, :],
                            in_=xT.rearrange("(k p) n -> p k n", p=128)[
                                :, 8 * half : 8 * half + 8, n0 : n0 + 512
                            ],
                        )
                        xh.append(t)
                    return xh

                # first x chunk before the weights: the first matmul chain
                # is gated on xh(0,0) + the low wqk half only.
                xh_first = load_x(0, 0)
                nc.sync.dma_start(
                    out=wqk_sb[:, 0:8, :],
                    in_=wqkT.rearrange("(k p) m -> p k m", p=128)[:, 0:8, :],
                )
                nc.sync.dma_start(
                    out=wqk_sb[:, 8:16, :],
                    in_=wqkT.rearrange("(k p) m -> p k m", p=128)[:, 8:16, :],
                )
                nc.sync.dma_start(
                    out=wv_sb[:, :, :],
                    in_=wvT.rearrange("(k p) m -> p k m", p=128),
                )
                nc.sync.dma_start(out=tri_sb[:, :], in_=tri[:, :])
                nc.vector.memset(ones_col[:, :], 1.0)

                for b in range(B):
                    qh_sb = persist.tile(
                        [128, 4, N], BF16, tag="qh", name=f"qh_b{b}"
                    )
                    vT_sb = persist.tile(
                        [128, 16, HPC * HD], BF16, tag="vT", name=f"vT_b{b}"
                    )
                    # ---------------- phase A: projection + RoPE ----------
                    for j in range(4):
                        n0 = b * N + 512 * j
                        xh = xh_first if (b, j) == (0, 0) else load_x(b, j)
                        tabt = []
                        for ti in range(4):
                            tt = tabp.tile([128, 512], BF16, tag=f"tab{ti}", name=f"tab{ti}_{b}_{j}")
                            nc.sync.dma_start(out=tt[:, :], in_=tabs[ti, :, n0 : n0 + 512])
                            tabt.append(tt)
                        for pair in (0, 2):
                            psA = pstp.tile([128, 512], F32, tag="pst", name=f"psA_{b}_{j}_{pair}")
                            psB = pstp.tile([128, 512], F32, tag="pst", name=f"psB_{b}_{j}_{pair}")
                            for mt, pst_ in ((pair, psA), (pair + 1, psB)):
                                for k in range(16):
                                    nc.tensor.matmul(
                                        pst_[:, :],
                                        lhsT=(wqk_sb[:, k, 128 * mt : 128 * mt + 128]),
                                        rhs=(xh[k // 8][:, k % 8, :]),
                                        start=(k == 0),
                                        stop=(k == 15),
                                    )
                            ci = 0 if pair == 0 else 2
                            t1 = ropep.tile([128, 512], BF16, tag="t1", name=f"t1_{b}_{j}_{pair}")
                            t2 = ropep.tile([128, 512], BF16, tag="t2", name=f"t2_{b}_{j}_{pair}")
                            t3 = ropep.tile([128, 512], BF16, tag="t3", name=f"t3_{b}_{j}_{pair}")
                            t4 = ropep.tile([128, 512], BF16, tag="t4", name=f"t4_{b}_{j}_{pair}")
                            nc.vector.tensor_tensor(t1[:, :], psA[:, :], tabt[ci][:, :], MUL)
                            nc.vector.tensor_tensor(t2[:, :], psB[:, :], tabt[ci + 1][:, :], MUL)
                            nc.vector.tensor_tensor(t3[:, :], psB[:, :], tabt[ci][:, :], MUL)
                            nc.vector.tensor_tensor(t4[:, :], psA[:, :], tabt[ci + 1][:, :], MUL)
                            sl = stagep.tile([128, 512], BF16, tag="sl", name=f"sl_{b}_{j}_{pair}")
                            sh = stagep.tile([128, 512], BF16, tag="sh", name=f"sh_{b}_{j}_{pair}")
                            nc.vector.tensor_tensor(sl[:, :], t1[:, :], t2[:, :], SUB)
                            nc.vector.tensor_tensor(sh[:, :], t3[:, :], t4[:, :], ADD)
                            # repack: per-head [lo;hi] tiles for full-contract
                            # scores.  base tile index: q -> 0, k -> 2.
                            base = 0 if pair == 0 else 2
                            cs = slice(512 * j, 512 * (j + 1))
                            nc.sync.dma_start(out=qh_sb[0:64, base, cs], in_=sl[0:64, :])
                            nc.sync.dma_start(out=qh_sb[0:64, base + 1, cs], in_=sl[64:128, :])
                            nc.sync.dma_start(out=qh_sb[64:128, base, cs], in_=sh[0:64, :])
                            nc.sync.dma_start(out=qh_sb[64:128, base + 1, cs], in_=sh[64:128, :])
                        for mt in range(4):
                            pv = povp.tile([128, 256], F32, tag="pov", name=f"psV_{b}_{j}_{mt}")
                            for k in range(16):
                                nc.tensor.matmul(
                                    pv[:, :],
                                    lhsT=(xh[k // 8][:, k % 8, 128 * mt : 128 * mt + 128]),
                                    rhs=(wv_sb[:, k, :]),
                                    start=(k == 0),
                                    stop=(k == 15),
                                )
                            nc.scalar.copy(vT_sb[:, 4 * j + mt, :], pv[:, :])

                    if b == 0:
                        # w_o cache fill: issued here so the 8MB transfer
                        # rides under b0's attention, clear of x loads.
                        nc.sync.dma_start(
                            out=wo_sb[:, :, :],
                            in_=woT.rearrange("(k p) d -> p k d", p=128),
                        )

                    # ---------------- phase B: attention ------------------
                    for h in range(HPC):
                        for j in range(4):
                            ov = povp.tile([128, 512], F32, tag="pov", name=f"ov_{b}_{h}_{j}")
                            rsum = psmallp.tile(
                                [1, 512], F32, tag="rsum", name=f"rsum_{b}_{h}_{j}"
                            )
                            nt = 4 * j + 4
                            for t in range(nt):
                                st = pstp.tile(
                                    [128, 512], F32, tag="pst", name=f"st_{b}_{h}_{j}_{t}"
                                )
                                nc.tensor.matmul(
                                    st[:, :],
                                    lhsT=(qh_sb[:, 2 + h, 128 * t : 128 * t + 128]),
                                    rhs=(qh_sb[:, h, 512 * j : 512 * (j + 1)]),
                                    start=True,
                                    stop=True,
                                )
                                pt = ptp.tile(
                                    [128, 512], BF16, tag="pt", name=f"pt_{b}_{h}_{j}_{t}"
                                )
                                nc.scalar.activation(pt[:, :], st[:, :], EXP)
                                if t // 4 == j:
                                    f0 = 128 * t - 512 * j
                                    nc.vector.tensor_tensor(
                                        pt[:, :], pt[:, :],
                                        tri_sb[:, 512 - f0 : 1024 - f0], MUL,
                                    )
                                nc.tensor.matmul(
                                    rsum[:, :],
                                    lhsT=ones_col[:, :],
                                    rhs=pt[:, :],
                                    start=(t == 0),
                                    stop=(t == nt - 1),
                                )
                                nc.tensor.matmul(
                                    ov[:, :],
                                    lhsT=(vT_sb[:, t, 128 * h : 128 * h + 128]),
                                    rhs=(pt[:, :]),
                                    start=(t == 0),
                                    stop=(t == nt - 1),
                                )
                            rinv = smallp.tile([1, 512], F32, tag="rinv", name=f"rinv_{b}_{h}_{j}")
                            nc.vector.reciprocal_approx_fast(rinv[:, :], rsum[:, :])
                            binv = smallp.tile(
                                [128, 512], F32, tag="binv", name=f"binv_{b}_{h}_{j}"
                            )
                            nc.gpsimd.partition_broadcast(binv[:, :], rinv[:, :])
                            ot = otsp.tile([128, 512], BF16, tag="ot", name=f"ot_{b}_{h}_{j}")
                            nc.vector.tensor_tensor(ot[:, :], ov[:, :], binv[:, :], MUL)
                            nc.sync.dma_start(
                                out=a2a_in[b][h][2 * j, :, :], in_=ot[:, 0:256]
                            )
                            nc.sync.dma_start(
                                out=a2a_in[b][h][2 * j + 1, :, :], in_=ot[:, 256:512]
                            )
                        # per-(batch,head) reshard: all but the last
                        # collective hide under subsequent compute; the last
                        # hides under phase C's first half.
                        nc.gpsimd.collective_compute(
                            "AllToAll",
                            mybir.AluOpType.bypass,
                            replica_groups=[list(range(NC))],
                            ins=[a2a_in[b][h].opt()],
                            outs=[a2a_out[b][h].opt()],
                        )

            # ---------------- phase C: o_proj ------------------------------
            # opin k-tile order is (h, src) -> woT rows are host-permuted to
            # match.  Output rows are staged in full-width tiles so each
            # 128-row block ships as one large DMA.
            with (
                tc.tile_pool(name="opin", bufs=1) as opinp,
                tc.tile_pool(name="outs", bufs=2) as outsp,
                tc.tile_pool(name="pc", bufs=4, space="PSUM") as pcp,
            ):
                for half in range(B):
                    opin = opinp.tile([128, 16, 256], BF16, tag=f"opin{half}", name=f"opin{half}")
                    for h in range(HPC):
                        nc.sync.dma_start(
                            out=opin[:, 8 * h : 8 * h + 8, :],
                            in_=a2a_out[half][h].rearrange("r p n -> p r n"),
                        )
                    for ns in range(2):
                        ost = outsp.tile([128, D], F32, tag="outs", name=f"os_{half}_{ns}")
                        for dc in range(4):
                            pc = pcp.tile([128, 512], F32, tag="pc", name=f"pc_{half}_{dc}_{ns}")
                            for k in range(16):
                                nc.tensor.matmul(
                                    pc[:, :],
                                    lhsT=(opin[:, k, 128 * ns : 128 * ns + 128]),
                                    rhs=(wo_sb[:, k, 512 * dc : 512 * (dc + 1)]),
                                    start=(k == 0),
                                    stop=(k == 15),
                                )
                            nc.scalar.copy(ost[:, 512 * dc : 512 * (dc + 1)], pc[:, :])
                        r0 = 256 * half + 128 * ns
                        nc.sync.dma_start(out=out[r0 : r0 + 128, :], in_=ost[:, :])
    nc.compile()
    return nc


def _host_prep(x, w_qkv, w_o):
    bf = ml_dtypes.bfloat16
    xT = np.ascontiguousarray(x.reshape(BN, D).T).astype(bf)
    # o_proj k-tile order on device is (h_local, src_core): head g lives at
    # slot 8*(g % 2) + g // 2.
    woT_n = np.asarray(w_o).T.reshape(H, HD, D)
    perm = [2 * s + hl for hl in range(HPC) for s in range(NC)]
    woT = np.ascontiguousarray(woT_n[perm].reshape(INNER, D)).astype(bf)

    inv_freq = 1.0 / (ROPE_BASE ** (np.arange(0, HD, 2, dtype=np.float32) / HD))
    ang = np.arange(N, dtype=np.float32)[:, None] * inv_freq[None, :]
    cos_h = np.cos(ang).T.astype(np.float32)      # [64, N]
    sin_h = np.sin(ang).T.astype(np.float32)      # [64, N] (magnitude)
    # duplicated for the two heads packed per 128-row block
    cos2 = np.concatenate([cos_h, cos_h], axis=0)  # [128, N]
    sin2 = np.concatenate([sin_h, sin_h], axis=0)
    cos_f = np.tile(cos2, (1, B))
    sin_f = np.tile(sin2, (1, B))
    scale = np.float32(1.0 / np.sqrt(HD))
    tabs = np.ascontiguousarray(
        np.stack([cos_f * scale, sin_f * scale, cos_f, sin_f], axis=0)
    ).astype(bf)

    p = np.arange(128)[:, None]
    c = np.arange(1024)[None, :]
    tri = (p <= c - 512).astype(bf)

    in_maps = []
    for core in range(NC):
        h0 = core * HPC
        rq = slice(h0 * HD, (h0 + HPC) * HD)
        rk = slice(INNER + h0 * HD, INNER + (h0 + HPC) * HD)
        rv = slice(2 * INNER + h0 * HD, 2 * INNER + (h0 + HPC) * HD)
        wq = w_qkv[rq].reshape(HPC, HD, D)
        wk = w_qkv[rk].reshape(HPC, HD, D)
        # row order per block: [h0_lo, h1_lo | h0_hi, h1_hi] for q then k
        wqkT = np.ascontiguousarray(
            np.concatenate(
                [wq[0, :64], wq[1, :64], wq[0, 64:], wq[1, 64:],
                 wk[0, :64], wk[1, :64], wk[0, 64:], wk[1, 64:]], axis=0
            ).T
        ).astype(bf)
        wvT = np.ascontiguousarray(w_qkv[rv].T).astype(bf)
        in_maps.append(
            dict(xT=xT, wqkT=wqkT, wvT=wvT, woT=woT, tabs=tabs, tri=tri)
        )
    return in_maps


def kernel(x, w_qkv, w_o, n_heads=None, head_dim=None, trace=False):
    global LAST_EXEC_NS, LAST_RESULTS
    x = np.asarray(x, dtype=np.float32)
    w_qkv = np.asarray(w_qkv, dtype=np.float32)
    w_o = np.asarray(w_o, dtype=np.float32)

    if "nc" not in _CACHE:
        _CACHE["nc"] = _build_program()
    nc = _CACHE["nc"]

    in_maps = _host_prep(x, w_qkv, w_o)
    res = None
    last_exc = None
    for attempt in range(4):
        try:
            res = bass_utils.run_bass_kernel_spmd(
                nc, in_maps, core_ids=list(range(NC)), trace=trace
            )
            break
        except Exception as e:  # transient compile_and_load / exec flakiness
            last_exc = e
            print(f"kernel attempt {attempt} failed: {e}", file=sys.stderr)
            time.sleep(5)
    if res is None:
        raise last_exc
    LAST_EXEC_NS = res.exec_time_ns
    LAST_RESULTS = res
    # core c returns [512, D]: rows 0:256 = batch0 rows 256c:256c+256,
    # rows 256:512 = batch1 rows 256c:256c+256.
    full = np.empty((B, N, D), dtype=np.float32)
    for c in range(NC):
        shard = res.results[c]["out"]
        full[0, 256 * c : 256 * c + 256] = shard[0:256]
        full[1, 256 * c : 256 * c + 256] = shard[256:512]
    return full
